# revision 1
# baseline (speedup 1.0000x reference)
"""Trainium2 Bass kernel for CSMultiHeadAttention (rotated cross-chunk MHA).

Sharding: data-parallel over batch (B=8) across the 8 NeuronCores; each core
computes one batch element end-to-end (no collectives).

Per-core dataflow (all matmuls bf16 inputs, fp32 PSUM accumulation):
  prep: x,W -> cast bf16 -> DRAM scratch -> DMA-transpose -> x^T, W^T in SBUF
  proj: Q^T = (Wq^T)^T stationary @ x^T moving (+bq), K^T likewise,
        V    = (x^T)^T stationary @ Wv^T moving (+bv) into head-strided
        V_aug layout [n, h, 65] with a ones column per head (gives softmax
        denominators for free during the AV matmul).
  attn (transposed layout, head-pair row-tiling on the 128x128 PE array):
        energy^T[k,q] = K^T_h.T @ Q^T_h  (d=64 contraction; heads 2j/2j+1
        occupy partition halves 0-63/64-127 -> concurrent row-tiled matmuls)
        att^T = exp(energy^T * 1/sqrt(E))  on ScalarE, PSUM->SBUF bf16
        out_aug^T[65,q] = V_aug.T @ att^T  (row 64 = softmax denominator)
        normalize: recip(denominator row) -> DMA partition-broadcast ->
        attout^T = out^T * recip  (bf16, already in the [e,n] layout the
        output projection needs as its stationary operand)
  proj2: y[n,f] = (attout^T).T stationary @ Wp^T moving + bp -> DRAM fp32
"""

import numpy as np

import concourse.bass as bass
import concourse.tile as tile
from concourse import bacc
from concourse import mybir
from concourse import bass_utils

F32 = mybir.dt.float32
BF16 = mybir.dt.bfloat16

B, S, E, H = 8, 3072, 512, 8
C = 3                # seq chunks
N = S // C           # 1024 tokens per chunk
D = E // H           # 64 head dim
P = 128              # partitions
ET = E // P          # 4 feature tiles
NT = N // P          # 8 token tiles per chunk
FREE = 512           # matmul moving free dim / PSUM bank (fp32)
NQ = N // FREE       # 2 q-halves per chunk
SCALE = float(1.0 / np.sqrt(np.float32(E)))
QSEL = [1, 2, 0]     # out chunk c uses Q of chunk QSEL[c]
KSEL = [2, 0, 1]     # ... and K,V of chunk KSEL[c]

_CACHE = {}


def _bcast_part(ap, nparts):
    """View a single-partition AP broadcast across nparts partitions."""
    return bass.AP(tensor=ap.tensor, offset=ap.offset,
                   ap=[[0, nparts]] + list(ap.ap)[1:])


def build_bass(repeats=1):
    nc = bacc.Bacc()
    x = nc.dram_tensor("x", [S, E], F32, kind="ExternalInput")
    W = {nm: nc.dram_tensor(nm, [C, E, E], F32, kind="ExternalInput")
         for nm in ("Wq", "Wk", "Wv", "Wp")}
    bias = {nm: nc.dram_tensor(nm, [C, E], F32, kind="ExternalInput")
            for nm in ("bq", "bk", "bv", "bp")}
    out = nc.dram_tensor("out", [S, E], F32, kind="ExternalOutput")

    with tile.TileContext(nc) as tc:
        for _rep in range(repeats):
            _emit_body(nc, tc, x, W, bias, out)
    nc.finalize()
    return nc


def _emit_body(nc, tc, x, W, bias, out):
    # Chunk prep order rotated so attention chunk 0 (which consumes Q of
    # chunk 1 and K/V of chunk 2) has its dependencies ready first.
    PREP_ORDER = [1, 2, 0]
    with (
        tc.tile_pool(name="dram", bufs=1, space="DRAM") as dram,
        tc.tile_pool(name="persist", bufs=1) as persist,
        tc.tile_pool(name="en_ps", bufs=2, space="PSUM") as en_ps,
        tc.tile_pool(name="av_ps", bufs=4, space="PSUM") as av_ps,
    ):
        # ---- bias tiles ----
        bqT, bkT, bv_bc, bp_bc = {}, {}, {}, {}
        for c in range(C):
            t_bq = persist.tile([P, ET], F32, name=f"bqT_{c}")
            nc.sync.dma_start(out=t_bq, in_=bias["bq"][c].rearrange(
                "(j p) -> p j", p=P))
            bqT[c] = t_bq
            t_bk = persist.tile([P, ET], F32, name=f"bkT_{c}")
            nc.sync.dma_start(out=t_bk, in_=bias["bk"][c].rearrange(
                "(j p) -> p j", p=P))
            bkT[c] = t_bk
            t_bv = persist.tile([P, E], BF16, name=f"bv_bc_{c}")
            nc.gpsimd.dma_start(out=t_bv,
                                in_=_bcast_part(bias["bv"][c:c + 1], P))
            bv_bc[c] = t_bv
            t_bp = persist.tile([P, E], F32, name=f"bp_bc_{c}")
            nc.sync.dma_start(out=t_bp, in_=_bcast_part(bias["bp"][c:c + 1], P))
            bp_bc[c] = t_bp

        WpT = [[persist.tile([P, E], BF16, name=f"WpT_{c}_{k}")
                for k in range(ET)] for c in range(C)]
        QT = [[persist.tile([P, N], BF16, name=f"QT_{c}_{j}")
               for j in range(ET)] for c in range(C)]
        KT = [[persist.tile([P, N], BF16, name=f"KT_{c}_{j}")
               for j in range(ET)] for c in range(C)]
        Vaug = [[persist.tile([P, H, D + 1], BF16, name=f"Vaug_{c}_{i}")
                 for i in range(NT)] for c in range(C)]

        ident = persist.tile([P, P], F32, name="ident")
        from concourse.masks import make_identity
        make_identity(nc, ident)

        xwpools = (tc.tile_pool(name="xT", bufs=1),
                   tc.tile_pool(name="wT", bufs=1))
        xpool, wpool = [p.__enter__() for p in xwpools]
        _ = None
        x_bf = dram.tile([S, E], BF16)
        xT = [[xpool.tile([P, N], BF16, name=f"xT_{c}_{k}")
               for k in range(ET)] for c in range(C)]
        WT = {nm: [[wpool.tile([P, E], BF16, name=f"{nm}T_{c}_{k}")
                    for k in range(ET)] for c in range(C)]
              for nm in ("Wq", "Wk", "Wv")}

        with tc.tile_pool(name="io", bufs=3) as io:
            # ---- phase A ----
            # x: cast to bf16 (on idle ScalarE), round-trip through DRAM
            # with DMA-transpose. W: load fp32 and transpose on the PE
            # (128x128 tiles into PSUM), evacuating with a bf16 cast.
            def prep_x(c):
                for quart in range(4):
                    r = c * N + quart * 2 * P
                    src3 = x[r:r + 2 * P, :].rearrange(
                        "(t p) e -> p t e", p=P)
                    dst3 = x_bf[r:r + 2 * P, :].rearrange(
                        "(t p) e -> p t e", p=P)
                    ld = io.tile([P, 2, E], F32, tag="ld",
                                 name=f"ld_x_{c}_{quart}", bufs=2)
                    nc.sync.dma_start(out=ld, in_=src3)
                    cs = io.tile([P, 2, E], BF16, tag="cs",
                                 name=f"cs_x_{c}_{quart}", bufs=2)
                    nc.scalar.copy(out=cs, in_=ld)
                    nc.sync.dma_start(out=dst3, in_=cs)
                for k in range(ET):
                    nc.sync.dma_start(
                        out=xT[c][k],
                        in_=x_bf[c * N:(c + 1) * N, k * P:(k + 1) * P],
                        transpose=True)

            def prep_w(nm, c):
                dsts = WpT if nm == "Wp" else WT[nm]
                wns = []
                for half in range(2):
                    wn = io.tile([P, 2, E], F32, tag="wn",
                                 name=f"wn_{nm}_{c}_{half}", bufs=3)
                    nc.sync.dma_start(
                        out=wn,
                        in_=W[nm][c, half * 2 * P:(half + 1) * 2 * P, :]
                        .rearrange("(t p) e -> p t e", p=P))
                    wns.append(wn)
                for k in range(ET):
                    ps = av_ps.tile([P, FREE], F32, tag="av",
                                    name=f"ps_w_{nm}_{c}_{k}")
                    for t in range(ET):
                        nc.tensor.transpose(
                            out=ps[:, t * P:(t + 1) * P],
                            in_=wns[t // 2][:, t % 2, k * P:(k + 1) * P],
                            identity=ident)
                    nc.vector.tensor_copy(out=dsts[c][k], in_=ps)

            for c in PREP_ORDER:
                for nm in ("Wq", "Wk", "Wv"):
                    prep_w(nm, c)
                prep_x(c)
            for c in PREP_ORDER:
                prep_w("Wp", c)

        # ---- phases B and C interleaved per output chunk ----
        def proj_qk(c, kind):
            wt = WT["Wq"] if kind == "q" else WT["Wk"]
            bt = bqT if kind == "q" else bkT
            dst = QT if kind == "q" else KT
            for j in range(ET):
                ps = en_ps.tile([P, N], F32, tag="en",
                                name=f"ps_{kind}_{c}_{j}")
                for qh in range(NQ):
                    for k in range(ET):
                        nc.tensor.matmul(
                            ps[:, qh * FREE:(qh + 1) * FREE],
                            lhsT=wt[c][k][:, j * P:(j + 1) * P],
                            rhs=xT[c][k][:, qh * FREE:(qh + 1) * FREE],
                            start=(k == 0), stop=(k == ET - 1))
                nc.vector.tensor_scalar_add(dst[c][j], ps, bt[c][:, j:j + 1])

        def proj_v(c):
            for i in range(NT):
                ps = av_ps.tile([P, FREE], F32, tag="av", name=f"ps_v_{c}_{i}")
                for k in range(ET):
                    nc.tensor.matmul(
                        ps,
                        lhsT=xT[c][k][:, i * P:(i + 1) * P],
                        rhs=WT["Wv"][c][k],
                        start=(k == 0), stop=(k == ET - 1))
                nc.vector.tensor_add(
                    out=Vaug[c][i][:, :, 0:D],
                    in0=ps.rearrange("p (h d) -> p h d", d=D),
                    in1=bv_bc[c].rearrange("p (h d) -> p h d", d=D))
                nc.vector.memset(Vaug[c][i][:, :, D:D + 1], 1.0)

        aoT_all = [[None] * ET for _ in range(C)]

        def attention(c):
            qc, kc = QSEL[c], KSEL[c]
            aoT = aoT_all[c]
            for j in range(ET):
                ao = aop.tile([P, N], BF16, name=f"aoT_{c}_{j}")
                aoT[j] = ao
                av_ts = [[av_ps.tile([D + 1, FREE], F32, tag="av",
                                     name=f"av_{c}_{j}_{hh}_{qh}")
                          for qh in range(NQ)] for hh in range(2)]
                for kt in range(NT):
                    en_ts = [en_ps.tile([P, N], F32, tag="en",
                                        name=f"en_{c}_{j}_{kt}_{hh}")
                             for hh in range(2)]
                    for qh in range(NQ):
                        for hh in range(2):
                            bp0 = D * hh
                            nc.tensor.matmul(
                                en_ts[hh][:, qh * FREE:(qh + 1) * FREE],
                                lhsT=KT[kc][j][bp0:bp0 + D,
                                               kt * P:(kt + 1) * P],
                                rhs=QT[qc][j][bp0:bp0 + D,
                                              qh * FREE:(qh + 1) * FREE],
                                start=True, stop=True)
                    at_ts = []
                    for hh in range(2):
                        at = attw.tile([P, N], BF16, tag="at",
                                       name=f"at_{c}_{j}_{kt}_{hh}")
                        nc.scalar.activation(
                            out=at, in_=en_ts[hh],
                            func=mybir.ActivationFunctionType.Exp,
                            scale=SCALE)
                        at_ts.append(at)
                    for qh in range(NQ):
                        for hh in range(2):
                            h = 2 * j + hh
                            nc.tensor.matmul(
                                av_ts[hh][qh],
                                lhsT=Vaug[kc][kt][:, h, :],
                                rhs=at_ts[hh][:, qh * FREE:(qh + 1) * FREE],
                                start=(kt == 0), stop=(kt == NT - 1))
                for hh in range(2):
                    # fast PSUM evac (frees the av slots); normalization
                    # happens from SBUF off the critical path
                    oc = normp.tile([D + 1, N], F32, tag="oc",
                                    name=f"oc_{c}_{j}_{hh}")
                    for qh in range(NQ):
                        nc.vector.tensor_copy(
                            out=oc[:, qh * FREE:(qh + 1) * FREE],
                            in_=av_ts[hh][qh])
                    recip = normp.tile([1, N], F32, tag="recip",
                                       name=f"rc_{c}_{j}_{hh}")
                    nc.vector.reciprocal(recip, oc[D:D + 1, :])
                    # partition-broadcast via DRAM bounce (SBUF source DMAs
                    # cannot have a zero partition step)
                    rd = dram.tile([1, N], F32, name=f"rd_{c}_{j}_{hh}")
                    nc.sync.dma_start(out=rd, in_=recip)
                    rb = normp.tile([D, N], F32, tag="rb",
                                    name=f"rb_{c}_{j}_{hh}")
                    nc.sync.dma_start(out=rb, in_=_bcast_part(rd, D))
                    nc.vector.tensor_mul(
                        ao[D * hh:D * hh + D, :], oc[0:D, :], rb)

        for c in range(C):
            proj_v(KSEL[c])
            proj_qk(QSEL[c], "q")
            proj_qk(KSEL[c], "k")
        for p in reversed(xwpools):
            p.__exit__(None, None, None)
        attn_pools = (
            tc.tile_pool(name="ao", bufs=1),
            tc.tile_pool(name="attw", bufs=6),
            tc.tile_pool(name="norm", bufs=4),
            tc.tile_pool(name="yout", bufs=4),
        )
        aop, attw, normp, yout = [p.__enter__() for p in attn_pools]
        for c in range(C):
            attention(c)

        # ---- phase D: output projection ----
        for c in range(C):
            aoT = aoT_all[c]
            for i in range(NT):
                psb = en_ps.tile([P, N], F32, tag="en", name=f"ps_y_{c}_{i}")
                ps = psb[:, 0:FREE]
                for k in range(ET):
                    nc.tensor.matmul(
                        ps,
                        lhsT=aoT[k][:, i * P:(i + 1) * P],
                        rhs=WpT[c][k],
                        start=(k == 0), stop=(k == ET - 1))
                y = yout.tile([P, E], F32, tag="y", name=f"y_{c}_{i}")
                nc.vector.tensor_add(out=y, in0=ps, in1=bp_bc[c])
                nc.sync.dma_start(
                    out=out[c * N + i * P:c * N + (i + 1) * P, :], in_=y)

        for p in reversed(attn_pools):
            p.__exit__(None, None, None)


def _make_runner(nc, n_cores):
    """Build a cached shard_map-jitted executor for the prebuilt Bass module
    (same lowering as bass2jax.run_bass_via_pjrt, but jitted once so repeated
    calls skip retracing/recompile)."""
    import jax
    from jax.sharding import Mesh, PartitionSpec
    from jax.experimental.shard_map import shard_map
    from concourse import mybir as _mybir
    from concourse.bass2jax import (
        _bass_exec_p, install_neuronx_cc_hook, partition_id_tensor)

    install_neuronx_cc_hook()

    partition_name = (nc.partition_id_tensor.name
                      if nc.partition_id_tensor else None)
    in_names, out_names, out_avals, zero_outs = [], [], [], []
    for alloc in nc.m.functions[0].allocations:
        if not isinstance(alloc, _mybir.MemoryLocationSet):
            continue
        name = alloc.memorylocations[0].name
        if alloc.kind == "ExternalInput":
            if name != partition_name:
                in_names.append(name)
        elif alloc.kind == "ExternalOutput":
            shape = tuple(alloc.tensor_shape)
            dtype = _mybir.dt.np(alloc.dtype)
            out_names.append(name)
            out_avals.append(jax.core.ShapedArray(shape, dtype))
            zero_outs.append(np.zeros(shape, dtype))
    n_params = len(in_names)
    all_names = in_names + out_names
    if partition_name is not None:
        all_names.append(partition_name)

    def _body(*args):
        operands = list(args)
        if partition_name is not None:
            operands.append(partition_id_tensor())
        return tuple(_bass_exec_p.bind(
            *operands,
            out_avals=tuple(out_avals),
            in_names=tuple(all_names),
            out_names=tuple(out_names),
            lowering_input_output_aliases=(),
            sim_require_finite=True,
            sim_require_nnan=True,
            nc=nc,
        ))

    devices = jax.devices()[:n_cores]
    mesh = Mesh(np.asarray(devices), ("core",))
    nin = n_params + len(out_names)
    sharded = jax.jit(
        shard_map(_body, mesh=mesh,
                  in_specs=(PartitionSpec("core"),) * nin,
                  out_specs=(PartitionSpec("core"),) * len(out_names),
                  check_rep=False),
        keep_unused=True)
    return sharded, in_names, out_names, out_avals, zero_outs


def get_runner():
    if "runner" not in _CACHE:
        if "nc" not in _CACHE:
            _CACHE["nc"] = build_bass()
        _CACHE["runner"] = _make_runner(_CACHE["nc"], B)
    return _CACHE["runner"]


def kernel(**inputs):
    if "nc" not in _CACHE:
        _CACHE["nc"] = build_bass()
    nc = _CACHE["nc"]

    x = np.ascontiguousarray(np.asarray(inputs["x"], dtype=np.float32))
    shared = {nm: np.ascontiguousarray(np.asarray(inputs[nm], np.float32))
              for nm in ("Wq", "bq", "Wk", "bk", "Wv", "bv", "Wp", "bp")}
    in_maps = [dict(shared, x=x[b]) for b in range(B)]
    res = bass_utils.run_bass_kernel_spmd(nc, in_maps, core_ids=list(range(B)))
    return np.stack([res.results[b]["out"] for b in range(B)], axis=0)

